# revision 5
# baseline (speedup 1.0000x reference)
"""Trainium2 Bass kernel for nn_Invert1_10: 16-step spiking recurrence on |x|.

Key math: the recurrence out(x) = scan(...) * sign(x) is elementwise, and since
z = ((v - T)/(|v|+1) > 0) <=> (v > T), the whole 16-step scan collapses to a
piecewise-constant function f(|x|) with 31 intervals, computable exactly (same
f32 semantics as the reference) by interval splitting on CPU from the
16-element h/d/T vectors.

Device evaluation per element (exact):
  out = (sum_k (delta_k/2) * ssign_k + C) * sign(x)
where ssign_k = Sign(2^40 * |x| + beta_k) in {-1,+1} via the ACT engine, with
beta_k = -fl(2^39*(b_k + s_k)) placed strictly between the scaled images of the
two adjacent f32 values b_k (last of interval k-1) and s_k (first of interval
k) -- so the indicator is exact for every f32 input, no boundary cases.

Engines: ACT does Abs + Sign(x) + 30 indicator Signs (1 elem/cyc/lane);
DVE does 30 fused mult-add accumulates (scalar_tensor_tensor) in two parallel
chains + final (acc+C)*sign.
"""

import os
import sys
import numpy as np

for _p in ("/opt/trn_rl_repo", "/opt/pypackages"):
    if _p not in sys.path and os.path.isdir(_p):
        sys.path.insert(0, _p)

N_CORES = 8
FULL_SHAPE = (16, 2048, 2048)
P = 128  # SBUF partitions
W = 2048  # tile free-dim width

_f32 = np.float32

LAST_EXEC_NS = None  # set by kernel() when KERNEL_TRACE=1


# ----------------------------------------------------------------------------
# CPU side: exact f32 interval splitting of the recurrence
# ----------------------------------------------------------------------------
def _apply_path(a, path):
    v = _f32(a)
    for hval in path:
        v = _f32(v - hval)
    return v


def _bisect_boundary(lo, hi, path, Tt):
    # largest f32 m in [lo,hi] with v(m) <= Tt; v monotone nondecreasing in a
    lo_i = int(_f32(lo).view(np.uint32))
    hi_i = int(_f32(hi).view(np.uint32))
    while hi_i - lo_i > 1:
        mid_i = (lo_i + hi_i) // 2
        m = np.uint32(mid_i).view(np.float32)
        if _apply_path(m, path) <= Tt:
            lo_i = mid_i
        else:
            hi_i = mid_i
    return np.uint32(lo_i).view(np.float32), np.uint32(hi_i).view(np.float32)


def _intervals(h, d, T):
    """Exact f32 intervals of a |-> out(a), a >= 0.

    Returns list of (lo, hi, value) with lo/hi inclusive f32 bounds."""
    h = np.asarray(h, np.float32)
    d = np.asarray(d, np.float32)
    T = np.asarray(T, np.float32)
    FMAX = np.finfo(np.float32).max
    ivs = [(_f32(0.0), _f32(FMAX), [], _f32(0.0), _f32(0.0))]
    for t in range(len(h)):
        nxt = []
        for (lo, hi, path, z, out) in ivs:
            path2 = path + [_f32(z * h[t])] if z == 1.0 else path
            vlo = _apply_path(lo, path2)
            vhi = _apply_path(hi, path2)
            Tt = T[t]
            if vlo > Tt:
                nxt.append((lo, hi, path2, _f32(1.0), _f32(out + d[t])))
            elif vhi <= Tt:
                nxt.append((lo, hi, path2, _f32(0.0), out))
            else:
                m0, m1 = _bisect_boundary(lo, hi, path2, Tt)
                nxt.append((lo, m0, path2, _f32(0.0), out))
                nxt.append((m1, hi, path2, _f32(1.0), _f32(out + d[t])))
        ivs = nxt
    merged = []
    for iv in ivs:
        if merged and merged[-1][2] == iv[4]:
            merged[-1] = (merged[-1][0], iv[1], merged[-1][2])
        else:
            merged.append((iv[0], iv[1], iv[4]))
    return merged


def _plan(h, d, T):
    """Build the device constants: betas (ACT Sign biases), half-deltas, C."""
    merged = _intervals(h, d, T)
    vals = np.array([m[2] for m in merged], dtype=np.float32)
    K = len(merged) - 1  # number of breakpoints
    deltas = np.empty(K, np.float64)
    betas = np.empty(K, np.float64)
    SCALE = float(2.0 ** 40)
    for k in range(K):
        b_k = merged[k][1]      # last f32 of interval k
        s_k = merged[k + 1][0]  # first f32 of interval k+1 (== nextafter(b_k))
        deltas[k] = float(vals[k + 1]) - float(vals[k])
        # beta strictly between -2^40*s_k and -2^40*b_k after f32 rounding
        betas[k] = -float(np.float32(2.0 ** 39 * (float(b_k) + float(s_k))))
    half = (deltas / 2.0).astype(np.float32)

    # C makes (sum_k half_k*ssign_k + C) == vals[j] in exact arithmetic:
    # C = vals[0] + sum(half)  (interval 0: all ssign = -1)
    C = np.float32(float(vals[0]) + float(np.sum(half.astype(np.float64))))

    # model the kernel's f32 accumulation (two chains, even/odd, then merged)
    def model(j):
        ss = np.where(np.arange(K) < j, np.float32(1.0), np.float32(-1.0))
        acc0 = np.float32(0.0)
        acc1 = np.float32(0.0)
        first0 = True
        first1 = True
        for k in range(K):
            t = np.float32(half[k] * ss[k])
            if k % 2 == 0:
                acc0 = t if first0 else np.float32(acc0 + t)
                first0 = False
            else:
                acc1 = t if first1 else np.float32(acc1 + t)
                first1 = False
        acc = np.float32(acc0 + acc1)
        return np.float32(acc + C)

    errs = np.array([float(model(j)) - float(vals[j]) for j in range(K + 1)])
    return {
        "K": K,
        "betas": betas.astype(np.float32),
        "half": half,
        "C": C,
        "vals": vals,
        "ends": np.array([m[1] for m in merged], np.float32),
        "max_model_err": float(np.abs(errs).max()),
        "scale": np.float32(SCALE),
    }


# ----------------------------------------------------------------------------
# Bass program
# ----------------------------------------------------------------------------
def _build_nc(plan, cols):
    import concourse.mybir as mybir
    from concourse import bacc
    from concourse.tile import TileContext

    f32 = mybir.dt.float32
    Alu = mybir.AluOpType
    Act = mybir.ActivationFunctionType

    K = plan["K"]
    betas = plan["betas"]
    half = plan["half"]
    C = float(plan["C"])
    SCALE = float(plan["scale"])

    nc = bacc.Bacc("TRN2", target_bir_lowering=False, debug=False,
                   num_devices=N_CORES)
    x_d = nc.dram_tensor("x", [P, cols], f32, kind="ExternalInput").ap()
    o_d = nc.dram_tensor("out", [P, cols], f32, kind="ExternalOutput").ap()

    # Register activation-bias constants (activation() requires biases as
    # const APs; same pattern as Bass.__init__'s register_const_ap).
    for k in range(K):
        val = float(betas[k])
        if (f32, val) in nc.const_aps.aps:
            continue
        t = nc.alloc_sbuf_tensor(f"const-beta-{k}", [P, 1], f32)
        nc.gpsimd.memset(t.ap(), val)
        nc.const_aps.aps[(f32, val)] = t.ap()

    n_tiles = cols // W
    with TileContext(nc) as tc:
        with (
            tc.tile_pool(name="xp", bufs=3) as xp,
            tc.tile_pool(name="ap_", bufs=2) as ap_,
            tc.tile_pool(name="sgp", bufs=2) as sgp,
            tc.tile_pool(name="ssp", bufs=6) as ssp,
            tc.tile_pool(name="accp", bufs=4) as accp,
            tc.tile_pool(name="op_", bufs=3) as op_,
        ):
            for j in range(n_tiles):
                sl = slice(j * W, (j + 1) * W)
                xt = xp.tile([P, W], f32, tag="x")
                nc.sync.dma_start(xt[:], x_d[:, sl])
                a = ap_.tile([P, W], f32, tag="a")
                nc.scalar.activation(a[:], xt[:], Act.Abs)
                sg = sgp.tile([P, W], f32, tag="sg")
                nc.scalar.activation(sg[:], xt[:], Act.Sign)
                acc0 = accp.tile([P, W], f32, tag="acc0")
                acc1 = accp.tile([P, W], f32, tag="acc1")
                for k in range(K):
                    ss = ssp.tile([P, W], f32, tag="ss")
                    nc.scalar.activation(ss[:], a[:], Act.Sign,
                                         bias=float(betas[k]), scale=SCALE)
                    acc = acc0 if k % 2 == 0 else acc1
                    if k < 2:
                        nc.vector.tensor_scalar(
                            acc[:], ss[:], float(half[k]), None, Alu.mult)
                    else:
                        nc.vector.scalar_tensor_tensor(
                            acc[:], ss[:], float(half[k]), acc[:],
                            Alu.mult, Alu.add)
                nc.vector.tensor_add(acc0[:], acc0[:], acc1[:])
                ot = op_.tile([P, W], f32, tag="o")
                nc.vector.scalar_tensor_tensor(
                    ot[:], acc0[:], C, sg[:], Alu.add, Alu.mult)
                nc.sync.dma_start(o_d[:, sl], ot[:])
    return nc


# ----------------------------------------------------------------------------
# PJRT runner (modeled on bass2jax.run_bass_via_pjrt, but keeps the jitted
# executable so warm runs can be timed; NTFF profiling is unavailable here)
# ----------------------------------------------------------------------------
_COMPILED = {}


def _get_runner(plan, cols):
    key = (cols, plan["betas"].tobytes(), plan["half"].tobytes())
    if key in _COMPILED:
        return _COMPILED[key]

    import jax
    import concourse.mybir as mybir
    from concourse import bass2jax
    from jax.experimental.shard_map import shard_map
    from jax.sharding import Mesh, PartitionSpec

    bass2jax.install_neuronx_cc_hook()
    nc = _build_nc(plan, cols)
    if not nc._finalized:
        nc.finalize()

    in_names, out_names, out_avals, zero_outs = [], [], [], []
    partition_name = (nc.partition_id_tensor.name
                      if nc.partition_id_tensor else None)
    for alloc in nc.m.functions[0].allocations:
        if not isinstance(alloc, mybir.MemoryLocationSet):
            continue
        name = alloc.memorylocations[0].name
        if alloc.kind == "ExternalInput":
            if name != partition_name:
                in_names.append(name)
        elif alloc.kind == "ExternalOutput":
            out_names.append(name)
            shape = tuple(alloc.tensor_shape)
            dtype = mybir.dt.np(alloc.dtype)
            out_avals.append(jax.core.ShapedArray(shape, dtype))
            zero_outs.append(np.zeros(shape, dtype))
    n_params = len(in_names)
    all_in_names = list(in_names) + list(out_names)
    if partition_name is not None:
        all_in_names.append(partition_name)

    def _body(*args):
        operands = list(args)
        if partition_name is not None:
            operands.append(bass2jax.partition_id_tensor())
        outs = bass2jax._bass_exec_p.bind(
            *operands,
            out_avals=tuple(out_avals),
            in_names=tuple(all_in_names),
            out_names=tuple(out_names),
            lowering_input_output_aliases=(),
            sim_require_finite=True,
            sim_require_nnan=True,
            nc=nc,
        )
        return tuple(outs)

    devices = jax.devices()[:N_CORES]
    mesh = Mesh(np.asarray(devices), ("core",))
    in_specs = (PartitionSpec("core"),) * (n_params + len(out_names))
    out_specs = (PartitionSpec("core"),) * len(out_names)
    fn = jax.jit(
        shard_map(_body, mesh=mesh, in_specs=in_specs, out_specs=out_specs,
                  check_rep=False),
        keep_unused=True,
    )
    runner = {
        "fn": fn, "mesh": mesh, "in_names": in_names,
        "out_names": out_names, "zero_outs": zero_outs,
    }
    _COMPILED[key] = runner
    return runner


def _run_full(runner, x):
    per = FULL_SHAPE[0] // N_CORES
    cols = (per * FULL_SHAPE[1] * FULL_SHAPE[2]) // P
    xg = np.ascontiguousarray(x).reshape(N_CORES * P, cols)
    z = runner["zero_outs"][0]
    zg = np.zeros((N_CORES * z.shape[0], *z.shape[1:]), z.dtype)
    (outg,) = runner["fn"](xg, zg)
    return np.asarray(outg).reshape(FULL_SHAPE)


def kernel(x, h, d, T):
    x = np.asarray(x)
    plan = _plan(h, d, T)
    assert plan["max_model_err"] <= 1e-6, plan["max_model_err"]
    per = FULL_SHAPE[0] // N_CORES
    cols = (per * FULL_SHAPE[1] * FULL_SHAPE[2]) // P
    runner = _get_runner(plan, cols)
    return _run_full(runner, x)


def bench(x, h, d, T, iters=5):
    """Warm on-device timing: returns (best_seconds, out)."""
    import time
    import jax
    from jax.sharding import NamedSharding, PartitionSpec

    x = np.asarray(x)
    plan = _plan(h, d, T)
    per = FULL_SHAPE[0] // N_CORES
    cols = (per * FULL_SHAPE[1] * FULL_SHAPE[2]) // P
    runner = _get_runner(plan, cols)
    sh = NamedSharding(runner["mesh"], PartitionSpec("core"))
    xg = jax.device_put(
        np.ascontiguousarray(x).reshape(N_CORES * P, cols), sh)
    z = runner["zero_outs"][0]
    zg = jax.device_put(
        np.zeros((N_CORES * z.shape[0], *z.shape[1:]), z.dtype), sh)
    fn = runner["fn"]
    out = fn(xg, zg)  # warm-up (compile)
    jax.block_until_ready(out)
    best = float("inf")
    for _ in range(iters):
        t0 = time.perf_counter()
        out = fn(xg, zg)
        jax.block_until_ready(out)
        best = min(best, time.perf_counter() - t0)
    return best, np.asarray(out[0]).reshape(FULL_SHAPE)
